# revision 11
# baseline (speedup 1.0000x reference)
"""LSTM kernel for Trainium2 (Bass/Tile), SPMD over 8 NeuronCores.

Problem: B=128, S=1024, D=256, H=512, C=10 LSTM; output = final hidden state
projected to C classes -> [B, C].

Sharding: data-parallel over batch (16 per core); weights replicated;
recurrence local per shard (no collectives).

Per-core program (two phases, one kernel launch):
  Phase 1: proj[t] = x_t @ [Wgx|Wix|Wfx|Wox ; b] for all t via full-PE GEMM
           (x stationary [128,128] tiles, W moving fp32r at 1 cyc/row),
           written to a DRAM scratch tensor.
  Phase 2: sequential recurrence. Per step: 16 matmuls (hT stationary
           [128,16], Wh moving N=512 fp32r) accumulate h@Wh into PSUM;
           DVE adds proj; ACT applies tanh/sigmoid per gate-pure 512-chunk;
           DVE cell update; PE-transposes h back into stationary hT form.
"""

import numpy as np

S, B, D, H, C = 1024, 128, 256, 512, 10
NCORES = 8
BC = B // NCORES          # batch per core
G4 = 4 * H                # fused gate width
NK_H = H // 128           # 4 K-tiles for h
NK_X = D // 128 + 1       # 2 K-tiles for x + 1 bias tile
CHUNK = 512               # PSUM-bank-sized gate chunk (one gate each: G,I,F,O)
NJ = G4 // CHUNK          # 4 chunks


def _build_nc(s_total: int):
    import concourse.bass as bass
    import concourse.mybir as mybir
    import concourse.tile as tile
    from concourse import bacc
    from concourse.masks import make_identity

    f32 = mybir.dt.float32
    f32r = mybir.dt.float32r
    AF = mybir.ActivationFunctionType
    OP = mybir.AluOpType

    m_tiles = s_total * BC // 128     # phase-1 M tiles (8 steps each)
    r_blocks = s_total // 4           # phase-2 proj DMA blocks

    nc = bacc.Bacc(
        "TRN2",
        target_bir_lowering=False,
        debug=False,
        enable_asserts=False,
        num_devices=NCORES,
    )

    xT_d = nc.dram_tensor("xT", [m_tiles, 128, NK_X, 128], f32r, kind="ExternalInput").ap()
    Wx_d = nc.dram_tensor("Wx", [NK_X, 128, G4], f32r, kind="ExternalInput").ap()
    Wh_d = nc.dram_tensor("Wh", [NK_H, 128, G4], f32r, kind="ExternalInput").ap()
    Wp_d = nc.dram_tensor("Wp", [NK_H, 128, C], f32r, kind="ExternalInput").ap()
    bp_d = nc.dram_tensor("bp", [BC, C], f32, kind="ExternalInput").ap()
    h0_d = nc.dram_tensor("h0", [128, NK_H * BC], f32r, kind="ExternalInput").ap()
    out_d = nc.dram_tensor("out", [BC, C], f32, kind="ExternalOutput").ap()

    with tile.TileContext(nc) as tc:
        with (
            tc.tile_pool(name="const", bufs=1) as const,
            tc.tile_pool(name="state", bufs=1) as state,
            tc.tile_pool(name="dram", bufs=1, space="DRAM") as dram,
        ):
            Wx_sb = const.tile([128, NK_X * G4], f32r)
            nc.sync.dma_start(
                Wx_sb[:].rearrange("p (k g) -> p k g", k=NK_X),
                Wx_d.rearrange("k p g -> p k g"),
            )
            Wh_sb = const.tile([128, NK_H * G4], f32r)
            nc.sync.dma_start(
                Wh_sb[:].rearrange("p (k g) -> p k g", k=NK_H),
                Wh_d.rearrange("k p g -> p k g"),
            )
            Wp_sb = const.tile([128, NK_H * C], f32r)
            nc.sync.dma_start(
                Wp_sb[:].rearrange("p (k c) -> p k c", k=NK_H),
                Wp_d.rearrange("k p c -> p k c"),
            )
            bp_sb = const.tile([BC, C], f32)
            nc.sync.dma_start(bp_sb[:], bp_d[:])
            ident = const.tile([BC, BC], f32)
            make_identity(nc, ident[:])

            # Recurrent state, ping-pong. hT is the transposed hidden state
            # [H-row, batch] packed as 4 K-tiles side by side: hT[:, 16k:16k+16].
            hT = [state.tile([128, NK_H * BC], f32r, tag=f"hT{i}", name=f"hT{i}") for i in range(2)]
            cs = [state.tile([BC, H], f32, tag=f"c{i}", name=f"c{i}") for i in range(2)]
            nc.sync.dma_start(hT[0][:], h0_d[:])
            nc.gpsimd.memset(cs[0][:], 0.0)

            # proj row index = 16*t + b (t = timestep, b = local batch)
            proj = dram.tile([s_total * BC, G4], f32)

            # ---------------- Phase 1: input projections ----------------
            with (
                tc.tile_pool(name="p1x", bufs=3) as p1x,
                tc.tile_pool(name="p1ps", bufs=2, space="PSUM") as p1ps,
                tc.tile_pool(name="p1st", bufs=3) as p1st,
            ):
                for m in range(m_tiles):
                    xt = p1x.tile([128, NK_X * 128], f32r)
                    nc.sync.dma_start(xt[:], xT_d[m].rearrange("p k c -> p (k c)"))
                    ps = p1ps.tile([128, G4], f32)
                    for j in range(NJ):
                        for k in range(NK_X):
                            nc.tensor.matmul(
                                ps[:, j * CHUNK:(j + 1) * CHUNK],
                                lhsT=xt[:, k * 128:(k + 1) * 128],
                                rhs=Wx_sb[:, k * G4 + j * CHUNK: k * G4 + (j + 1) * CHUNK],
                                start=(k == 0),
                                stop=(k == NK_X - 1),
                            )
                    st = p1st.tile([128, G4], f32)
                    for j in range(NJ):
                        src = ps[:, j * CHUNK:(j + 1) * CHUNK]
                        dst = st[:, j * CHUNK:(j + 1) * CHUNK]
                        if j % 2 == 0:
                            nc.vector.tensor_copy(dst, src)
                        else:
                            nc.scalar.copy(dst, src)
                    # m-tile covers steps 8m..8m+7 = proj blocks 2m, 2m+1;
                    # sbuf partition p = (t-8m)*16 + b matches (Blk s b) order.
                    nc.sync.dma_start(proj[128 * m:128 * (m + 1), :], st[:])

            # ---------------- Phase 2: recurrence ----------------
            with (
                tc.tile_pool(name="p2pj", bufs=2) as p2pj,
                tc.tile_pool(name="p2ps", bufs=1, space="PSUM") as p2ps,
                tc.tile_pool(name="p2tr", bufs=2, space="PSUM") as p2tr,
                tc.tile_pool(name="p2g", bufs=2) as p2g,
                tc.tile_pool(name="p2t", bufs=2) as p2t,
                tc.tile_pool(name="p2o", bufs=1, space="PSUM") as p2o,
            ):
                for r in range(r_blocks):
                    pj = p2pj.tile([BC, 4 * G4], f32)
                    nc.sync.dma_start(
                        pj[:].rearrange("b (s f) -> b s f", s=4),
                        proj[64 * r:64 * (r + 1), :].rearrange("(s b) f -> b s f", s=4),
                    )
                    for sidx in range(4):
                        t = 4 * r + sidx
                        cur, nxt = t % 2, (t + 1) % 2
                        ps = p2ps.tile([BC, G4], f32)
                        for j in range(NJ):
                            for k in range(NK_H):
                                nc.tensor.matmul(
                                    ps[:, j * CHUNK:(j + 1) * CHUNK],
                                    lhsT=hT[cur][:, k * BC:(k + 1) * BC],
                                    rhs=Wh_sb[:, k * G4 + j * CHUNK: k * G4 + (j + 1) * CHUNK],
                                    start=(k == 0),
                                    stop=(k == NK_H - 1),
                                )
                        gates = []
                        for j in range(NJ):
                            pre = p2t.tile([BC, CHUNK], f32, tag="pre", name="pre")
                            nc.vector.scalar_tensor_tensor(
                                pre[:],
                                ps[:, j * CHUNK:(j + 1) * CHUNK],
                                1.0,
                                pj[:, sidx * G4 + j * CHUNK: sidx * G4 + (j + 1) * CHUNK],
                                op0=OP.mult,
                                op1=OP.add,
                            )
                            gate = p2g.tile([BC, CHUNK], f32, tag=f"gate{j}", name=f"gate{j}")
                            nc.scalar.activation(
                                gate[:], pre[:],
                                AF.Tanh if j == 0 else AF.Sigmoid,
                            )
                            gates.append(gate)
                        g_, i_, f_, o_ = gates
                        gi = p2t.tile([BC, H], f32, tag="gi", name="gi")
                        nc.vector.tensor_mul(gi[:], g_[:], i_[:])
                        cn = cs[nxt]
                        nc.vector.tensor_mul(cn[:], cs[cur][:], f_[:])
                        nc.vector.tensor_add(cn[:], cn[:], gi[:])
                        th = p2t.tile([BC, H], f32, tag="th", name="th")
                        nc.scalar.activation(th[:], cn[:], AF.Tanh)
                        hn = p2t.tile([BC, H], f32, tag="hn", name="hn")
                        nc.vector.tensor_mul(hn[:], th[:], o_[:])
                        tr = p2tr.tile([128, NK_H * BC], f32)
                        for k in range(NK_H):
                            nc.tensor.transpose(
                                tr[:, k * BC:(k + 1) * BC],
                                hn[:, k * 128:(k + 1) * 128],
                                ident[:],
                            )
                        nc.vector.tensor_copy(hT[nxt][:], tr[:])

                # Final projection: out = h_S @ Wp + bp
                fin = s_total % 2
                pso = p2o.tile([BC, C], f32)
                for k in range(NK_H):
                    nc.tensor.matmul(
                        pso[:],
                        lhsT=hT[fin][:, k * BC:(k + 1) * BC],
                        rhs=Wp_sb[:, k * C:(k + 1) * C],
                        start=(k == 0),
                        stop=(k == NK_H - 1),
                    )
                res = p2g.tile([BC, C], f32, tag="res", name="res")
                nc.vector.tensor_add(res[:], pso[:], bp_sb[:])
                nc.sync.dma_start(out_d[:], res[:])

    nc.compile()
    return nc


def _prep_core_inputs(x, Wx_all, b_all, Wh_all, Wp, bp, core, s_total):
    """Build per-core numpy input map. x: [B, S, D] full batch."""
    m_tiles = s_total * BC // 128
    b0 = core * BC
    xc = x[b0:b0 + BC, :s_total, :]                     # [BC, s, D]
    # xT_host[m, p, kx, c]: stationary tiles; col c = (t - 8m)*16 + b
    a = np.ascontiguousarray(xc.transpose(2, 1, 0))     # [D, s, BC]
    a = a.reshape(D // 128, 128, m_tiles, 8, BC)        # [kx, p, m, t8, b]
    a = a.transpose(2, 1, 0, 3, 4).reshape(m_tiles, 128, D // 128, 128)
    xT = np.zeros((m_tiles, 128, NK_X, 128), dtype=np.float32)
    xT[:, :, :D // 128, :] = a
    xT[:, 0, NK_X - 1, :] = 1.0                          # bias ones-row
    return {"xT": np.ascontiguousarray(xT)}


def _prep_shared_inputs(Wgx, Wix, Wfx, Wox, Wgh, Wih, Wfh, Woh, bg, bi, bf, bo, Wph, bp):
    Wx_all = np.concatenate([Wgx, Wix, Wfx, Wox], axis=1).astype(np.float32)  # [D, G4]
    b_all = np.concatenate([bg, bi, bf, bo]).astype(np.float32)               # [G4]
    Wh_all = np.concatenate([Wgh, Wih, Wfh, Woh], axis=1).astype(np.float32)  # [H, G4]

    Wx = np.zeros((NK_X, 128, G4), dtype=np.float32)
    Wx[:D // 128] = Wx_all.reshape(D // 128, 128, G4)
    Wx[NK_X - 1, 0, :] = b_all                           # bias row (pairs with ones-row)
    Wh = np.ascontiguousarray(Wh_all.reshape(NK_H, 128, G4))
    Wp = np.ascontiguousarray(Wph.reshape(NK_H, 128, C).astype(np.float32))
    bpr = np.broadcast_to(bp.astype(np.float32), (BC, C)).copy()
    return Wx, Wh, Wp, bpr, Wx_all, b_all, Wh_all


_NC_CACHE = {}


def _get_nc(s_total):
    if s_total not in _NC_CACHE:
        _NC_CACHE[s_total] = _build_nc(s_total)
    return _NC_CACHE[s_total]


def kernel(x, Wgx, Wix, Wfx, Wox, Wgh, Wih, Wfh, Woh, bg, bi, bf, bo, Wph, bp,
           _s_total=S, _trace=False, _trace_kwargs=None):
    from concourse import bass_utils

    x = np.asarray(x, dtype=np.float32)
    args = [np.asarray(a, dtype=np.float32) for a in
            (Wgx, Wix, Wfx, Wox, Wgh, Wih, Wfh, Woh, bg, bi, bf, bo, Wph, bp)]
    Wx, Wh, Wp, bpr, Wx_all, b_all, Wh_all = _prep_shared_inputs(*args)

    nc = _get_nc(_s_total)
    in_maps = []
    for core in range(NCORES):
        m = _prep_core_inputs(x, Wx_all, b_all, Wh_all, Wp, bpr, core, _s_total)
        m.update({"Wx": Wx, "Wh": Wh, "Wp": Wp, "bp": bpr,
                  "h0": np.zeros((128, NK_H * BC), np.float32)})
        in_maps.append(m)

    kw = {}
    if _trace:
        kw["trace"] = True
        kw.update(_trace_kwargs or {})
    res = bass_utils.run_bass_kernel_spmd(nc, in_maps, core_ids=list(range(NCORES)), **kw)
    out = np.concatenate([res.results[c]["out"] for c in range(NCORES)], axis=0)
    if _trace:
        kernel._last_results = res
    return out


def _sim_selftest(s_total=16):
    """CoreSim numerics check on one core vs numpy LSTM (no hardware)."""
    from concourse.bass_interp import CoreSim

    rng = np.random.default_rng(0)
    x = rng.standard_normal((B, s_total, D), dtype=np.float32)
    mk = lambda *s: (rng.standard_normal(s, dtype=np.float32) * 0.06)
    Wgx, Wix, Wfx, Wox = (mk(D, H) for _ in range(4))
    Wgh, Wih, Wfh, Woh = (mk(H, H) for _ in range(4))
    bg, bi, bf, bo = (rng.standard_normal(H).astype(np.float32) * 0.05 for _ in range(4))
    Wph = mk(H, C)
    bp = rng.standard_normal(C).astype(np.float32) * 0.05

    def ref_np(xc):
        sig = lambda v: 1.0 / (1.0 + np.exp(-v))
        h = np.zeros((xc.shape[0], H), np.float32)
        c = np.zeros((xc.shape[0], H), np.float32)
        for t in range(s_total):
            xt = xc[:, t, :]
            g = np.tanh(xt @ Wgx + bg + h @ Wgh)
            i = sig(xt @ Wix + bi + h @ Wih)
            f = sig(xt @ Wfx + bf + h @ Wfh)
            o = sig(xt @ Wox + bo + h @ Woh)
            c = g * i + c * f
            h = np.tanh(c) * o
        return h @ Wph + bp

    args = (Wgx, Wix, Wfx, Wox, Wgh, Wih, Wfh, Woh, bg, bi, bf, bo, Wph, bp)
    Wx, Wh, Wp, bpr, Wx_all, b_all, Wh_all = _prep_shared_inputs(*args)
    nc = _build_nc(s_total)

    core = 1
    m = _prep_core_inputs(x, Wx_all, b_all, Wh_all, Wp, bpr, core, s_total)
    m.update({"Wx": Wx, "Wh": Wh, "Wp": Wp, "bp": bpr,
              "h0": np.zeros((128, NK_H * BC), np.float32)})

    sim = CoreSim(nc)
    for k, v in m.items():
        sim.tensor(k)[:] = v
    sim.simulate(check_with_hw=False)
    got = np.array(sim.tensor("out"))
    want = ref_np(x[core * BC:(core + 1) * BC])
    err = np.abs(got - want).max() / max(np.abs(want).max(), 1e-6)
    print(f"selftest S={s_total}: rel err {err:.3e}")
    assert err < 2e-2, err
    return err


if __name__ == "__main__":
    _sim_selftest(16)
